# revision 11
# baseline (speedup 1.0000x reference)
"""Multi-head attention (COAMultiHeadAttention) on 8 Trainium2 NeuronCores.

Sharding: batch x head-group. Core c (0..7) handles batch b = c//4 and head
group g = c%4 (4 of 16 heads, a 256-wide slice of the 1024-dim model).

v2 schedule (vs the 237us baseline): the ScalarE exp stream is the wall
(~143us dense), so everything else is packed around it:
  - DMA order w -> xk -> xq -> xv(fp8, token-major) lets K/Q projections
    stream against chunk arrival; the exp stream starts at ~38us (was 64).
  - Q^T/K^T are written in fp8e4 with head-dim split 32x2 across
    (partition, free) so QK^T runs in DoubleRow perf mode: half the PE
    cycles per score tile.
  - ~40 of the 128 exp tiles run on the idle VectorE as a Schraudolph
    bitcast exp (one fused mult+add -> int16 bits == bf16 exp), cutting the
    ScalarE wall to ~98us.
  - V-proj chunks 6..15 stream inside the first attention block through a
    1-bank PSUM pool; PV lags ~9 steps behind QK/exp until the att
    accumulators' banks free up (PSUM is exactly 8 banks: st 4 + att 4).
  - Block tails bounce softmax denominators through DRAM (hidden under the
    next block); the final block normalizes via a PE ones-broadcast of the
    reciprocal row so the exposed tail chain is short.
  - Output projection chunks + their DMAs run after the last norm,
    evac alternating ScalarE/VectorE.
Host sums the 4 partials per batch in fp32 and adds bo.
"""

import os

import ml_dtypes
import numpy as np

import concourse.bass as bass  # noqa: F401  (AP types resolve through this import)
import concourse.mybir as mybir
import concourse.tile as tile
from concourse import bacc, bass_utils

F32 = mybir.dt.float32
BF16 = mybir.dt.bfloat16
F8 = mybir.dt.float8e4
I16 = mybir.dt.int16
AT = mybir.ActivationFunctionType
ALU = mybir.AluOpType
DR = mybir.MatmulPerfMode.DoubleRow

B = 2
T = 2048
D = 1024
N_HEADS = 16
HEAD_DIM = 64
N_CORES = 8
S = 256            # per-core slice of the model dim (4 heads)
NHL = 4            # heads per core
P = 128
DC = D // P        # 8 contraction chunks for the projections
TC = T // P        # 16 token chunks
QH = 1024          # q-block width (PSUM-bank limited)
SCALE = 1.0 / np.sqrt(HEAD_DIM)

# Schraudolph exp: bf16 bits of exp(s*SCALE) ~= int16(s*A + B).
SCH_A = float(SCALE * np.log2(np.e) * 128.0)
SCH_B = float(128.0 * (127.0 - 0.0573))

_CACHE = {}
LAST_STATS = {}


def _dve_tile(hb, i):
    """Which exp tiles run on VectorE (Schraudolph): ~40 of 128."""
    return (hb == 1 and i % 2 == 1) or (hb == 0 and i % 8 == 2)


def _build_program():
    nc = bacc.Bacc("TRN2", target_bir_lowering=False, debug=False)

    xk_d = nc.dram_tensor("xk", [P, DC, T], BF16, kind="ExternalInput").ap()
    xq_d = nc.dram_tensor("xq", [P, 2, DC, QH], BF16, kind="ExternalInput").ap()
    xv_d = nc.dram_tensor("xv", [P, TC, DC, P], F8, kind="ExternalInput").ap()
    wqt_d = nc.dram_tensor("wqt", [P, DC, 2, P], BF16, kind="ExternalInput").ap()
    wkt_d = nc.dram_tensor("wkt", [P, DC, 2, P], BF16, kind="ExternalInput").ap()
    wvt_d = nc.dram_tensor("wvt", [P, DC, S], BF16, kind="ExternalInput").ap()
    bq_d = nc.dram_tensor("bq", [P, 2], F32, kind="ExternalInput").ap()
    bk_d = nc.dram_tensor("bk", [P, 2], F32, kind="ExternalInput").ap()
    bv_d = nc.dram_tensor("bv", [P, NHL, HEAD_DIM], F32, kind="ExternalInput").ap()
    wot_d = nc.dram_tensor("wot", [P, 2, D], BF16, kind="ExternalInput").ap()
    out_d = nc.dram_tensor("out_part", [TC, P, D], BF16, kind="ExternalOutput").ap()
    sums_d = nc.dram_tensor("sums_scr", [NHL, T], F32).ap()
    rsums_d = nc.dram_tensor("rsums_scr", [NHL, T], F32).ap()

    with tile.TileContext(nc) as tc:
        _body(tc, xk_d, xq_d, xv_d, wqt_d, wkt_d, wvt_d,
              bq_d, bk_d, bv_d, wot_d, out_d, sums_d, rsums_d)
    nc.compile()
    return nc


def _body(tc, xk_d, xq_d, xv_d, wqt_d, wkt_d, wvt_d, bq_d, bk_d, bv_d, wot_d,
          out_d, sums_d, rsums_d):
    nc = tc.nc

    from contextlib import ExitStack
    with ExitStack() as ctx:
        pers = ctx.enter_context(tc.tile_pool(name="pers", bufs=1))
        # (p64, mh, j, t): head 2*mh+hb at partitions 32*hb..32*hb+32
        # (PE matmul base partitions are limited to {0, 32, 64})
        qt_sb = pers.tile([64, 2, 2, T], F8, tag="qt")
        kt_sb = pers.tile([64, 2, 2, T], F8, tag="kt")
        v_sb = pers.tile([P, TC, NHL, 68], BF16, tag="v")
        attn_sb = pers.tile([P, 2, T], BF16, tag="attn")
        wot_sb = pers.tile([P, 2, D], BF16, tag="wot")
        bq_sb = pers.tile([P, 2], F32, tag="bq")
        bk_sb = pers.tile([P, 2], F32, tag="bk")
        bv_sb = pers.tile([P, NHL, HEAD_DIM], F32, tag="bv")
        zero_sb = pers.tile([P, 1], F32, tag="zero")
        scr_sb = pers.tile([P, 1], F32, tag="scr")
        ones_sb = pers.tile([1, HEAD_DIM], F32, tag="ones")
        # xv/wv live through block 1 (V chunks stream in-block)
        wv_sb = pers.tile([P, DC, S], BF16, tag="wv")
        xv_sb = pers.tile([P, TC, DC, P], F8, tag="xv")

        # ---------------- Phase A: K, Q projections + V chunks 0..5 --------
        def v_chunk(pool, t16):
            """Project V token-chunk t16 into v_sb (fp8 x stationary)."""
            ps = pool.tile([P, 512], F32, tag="pj", name=f"vps{t16}")
            for c in range(DC):
                nc.tensor.matmul(
                    ps[:, 0:S],
                    lhsT=xv_sb[:, t16, c, :],
                    rhs=wv_sb[:, c, :],
                    start=(c == 0), stop=(c == DC - 1))
            nc.vector.tensor_tensor(
                v_sb[:, t16, :, 0:64],
                ps[:, 0:S].rearrange("p (h x) -> p h x", h=NHL),
                bv_sb[:], op=ALU.add)

        # xk/xq/wq/wk are only needed for phase A; their pool closes before
        # the attention pools open, freeing ~72KB/partition of SBUF.
        with tc.tile_pool(name="xw", bufs=1) as xw, \
             tc.tile_pool(name="pjps", bufs=4, space="PSUM") as pjps:
            wq_sb = xw.tile([P, DC, 2, P], BF16, tag="wq")
            wk_sb = xw.tile([P, DC, 2, P], BF16, tag="wk")
            xk_sb = xw.tile([P, DC, T], BF16, tag="xk")
            xq_sb = xw.tile([P, 2, DC, QH], BF16, tag="xq")

            # Small tensors first so warm-up matmuls can start early, then
            # the projection-critical stream xk -> xq -> xv.
            nc.sync.dma_start(bq_sb[:], bq_d[:])
            nc.sync.dma_start(bk_sb[:], bk_d[:])
            nc.sync.dma_start(bv_sb[:], bv_d[:])
            nc.sync.dma_start(wv_sb[:], wvt_d[:])
            for c in range(DC):
                nc.sync.dma_start(wk_sb[:, c], wkt_d[:, c])
                nc.sync.dma_start(wq_sb[:, c], wqt_d[:, c])
            for c in range(DC):
                nc.sync.dma_start(xk_sb[:, c], xk_d[:, c])
            for h in range(2):
                for c in range(DC):
                    nc.sync.dma_start(xq_sb[:, h, c], xq_d[:, h, c])
            for t16 in range(TC):
                nc.sync.dma_start(xv_sb[:, t16], xv_d[:, t16])
            nc.sync.dma_start(wot_sb[:], wot_d[:])

            nc.vector.memset(zero_sb[:], 0.0)
            nc.vector.memset(ones_sb[:], 1.0)
            # Preload the exp table set (~2.7us) so the first real exp
            # doesn't stall the attention pipeline.
            nc.scalar.activation(scr_sb[:], zero_sb[:], AT.Exp,
                                 bias=zero_sb[:, 0:1], scale=1.0)
            # ones column for the P~V denominator trick
            nc.vector.memset(v_sb[:, :, :, 64:65], 1.0)
            # Warm-up matmuls: ramp the PE pstate while DMAs stream.
            wtile = pjps.tile([P, 512], F32, tag="pj", name="warm")
            for _ in range(12):
                nc.tensor.matmul(wtile[:, 0:S], lhsT=wv_sb[:, 0, 0:P],
                                 rhs=wv_sb[:, 0, :], start=True, stop=True)

            # K projection: PSUM partition p holds slice-dim
            # tau_j(p) = 64*(p//32) + j*32 + p%32; output fp8 for DoubleRow.
            # Evac splits rows 0:64 (pair mh=0) / 64:128 (mh=1, partition
            # shift down by 64 — legal for DVE).
            def proj_kq(w_sb, b_sb, x_rhs, dst, j, nlist, nw):
                tiles = [pjps.tile([P, 512], F32, tag="pj", name=f"pj{j}{n}")
                         for n in nlist]
                for c in range(DC):
                    for ti, n in enumerate(nlist):
                        nc.tensor.matmul(
                            tiles[ti][:, 0:nw],
                            lhsT=w_sb[:, c, j, :],
                            rhs=x_rhs(c, n, nw),
                            start=(c == 0), stop=(c == DC - 1))
                for ti, n in enumerate(nlist):
                    ns = slice(n * nw, (n + 1) * nw)
                    for mh in range(2):
                        rows = slice(mh * 64, mh * 64 + 64)
                        nc.vector.tensor_scalar(
                            dst[0:64, mh, j, ns], tiles[ti][rows, 0:nw],
                            b_sb[rows, j:j + 1], None, op0=ALU.add)

            def xk_rhs(c, n, nw):
                return xk_sb[:, c, n * nw:(n + 1) * nw]

            proj_kq(wk_sb, bk_sb, xk_rhs, kt_sb, 0, [0, 1, 2, 3], 512)
            proj_kq(wk_sb, bk_sb, xk_rhs, kt_sb, 1, [0, 1, 2, 3], 512)

            def xq_rhs_h(h):
                def f(c, n, nw):
                    # n is global over T; the SBUF half h holds local cols
                    return xq_sb[:, h, c, (n - 2 * h) * nw:(n - 2 * h + 1) * nw]
                return f

            # Q: n-index is global over T (h picks the half)
            for j in range(2):
                proj_kq(wq_sb, bq_sb, xq_rhs_h(0), qt_sb, j, [0, 1], 512)
            for j in range(2):
                proj_kq(wq_sb, bq_sb, xq_rhs_h(1), qt_sb, j, [2, 3], 512)

            # V chunks 0..5 last: xv streams in after xq
            for t16 in range(6):
                v_chunk(pjps, t16)

        # ---------------- Phase B: attention ----------------
        # Blocks jh-major-ish: (mh, jh) in order (0,0),(1,0),(0,1),(1,1).
        # QK^T in fp8 DoubleRow: head h = 2*mh+hb lives at partitions
        # 32h..32h+32 of kt/qt with the other 32 head-dims in the j free dim.
        blocks = [(0, 0), (1, 0), (0, 1), (1, 1)]

        with tc.tile_pool(name="stp", bufs=2, space="PSUM") as stp, \
             tc.tile_pool(name="ptp", bufs=24) as ptp, \
             tc.tile_pool(name="asb", bufs=2) as asbp, \
             tc.tile_pool(name="brd", bufs=2) as brdp, \
             tc.tile_pool(name="rcp", bufs=2) as rcpp:
            pending_pv = []
            att_tiles = {}   # bi -> (att_A, att_B)
            attp_box = []

            def emit_qk_exp(mh, jh, i):
                q0 = jh * QH
                st_A = stp.tile([P, QH], F32, tag="st", name="st_A")
                st_B = stp.tile([P, QH], F32, tag="st", name="st_B")
                for hb, st in ((0, st_A), (1, st_B)):
                    hp = slice(32 * hb, 32 * hb + 32)
                    for n in range(2):
                        ns = slice(n * 512, (n + 1) * 512)
                        qs = slice(q0 + n * 512, q0 + (n + 1) * 512)
                        nc.tensor.matmul(
                            st[:, ns],
                            lhsT=kt_sb[hp, mh, :, i * P:(i + 1) * P],
                            rhs=qt_sb[hp, mh, :, qs],
                            start=True, stop=True, perf_mode=DR)
                pts = []
                for hb, st in ((0, st_A), (1, st_B)):
                    pt = ptp.tile([P, QH], BF16, tag="pt", name=f"pt{hb}")
                    if _dve_tile(hb, i):
                        nc.vector.tensor_scalar(
                            pt[:].bitcast(I16), st[:], SCH_A, SCH_B,
                            op0=ALU.mult, op1=ALU.add)
                    else:
                        nc.scalar.activation(pt[:], st[:], AT.Exp,
                                             bias=zero_sb[:, 0:1],
                                             scale=float(SCALE))
                    pts.append(pt)
                return pts

            def emit_pv():
                bi, mh, i, pt_A, pt_B = pending_pv.pop(0)
                if bi not in att_tiles:
                    attp = attp_box[0]
                    att_tiles[bi] = (
                        attp.tile([65, QH], F32, tag="att", name="att_A"),
                        attp.tile([65, QH], F32, tag="att", name="att_B"))
                att_A, att_B = att_tiles[bi]
                for n in range(2):
                    ns = slice(n * 512, (n + 1) * 512)
                    nc.tensor.matmul(
                        att_A[:, ns], lhsT=v_sb[:, i, 2 * mh, 0:65],
                        rhs=pt_A[:, ns], start=(i == 0), stop=(i == TC - 1))
                    nc.tensor.matmul(
                        att_B[:, ns], lhsT=v_sb[:, i, 2 * mh + 1, 0:65],
                        rhs=pt_B[:, ns], start=(i == 0), stop=(i == TC - 1))

            def emit_block_tail(bi, mh, jh):
                """Evacuate + normalize via DRAM-bounced reciprocal bcast."""
                att_A, att_B = att_tiles.pop(bi)
                q0 = jh * QH
                attsbs = []
                for hb, att_ps in ((0, att_A), (1, att_B)):
                    attsb = asbp.tile([65, QH], F32, tag="attsb",
                                      name=f"attsb{hb}")
                    nc.vector.tensor_copy(attsb[:], att_ps[:])
                    attsbs.append(attsb)
                for hb, attsb in ((0, attsbs[0]), (1, attsbs[1])):
                    h = 2 * mh + hb
                    ph = hb * 64
                    nc.sync.dma_start(sums_d[h:h + 1, q0:q0 + QH],
                                      attsb[64:65, :])
                    sp = rcpp.tile([P, QH // P], F32, tag="sp")
                    nc.sync.dma_start(
                        sp[:], sums_d[h, q0:q0 + QH].rearrange(
                            "(p f) -> p f", p=P))
                    rp = rcpp.tile([P, QH // P], F32, tag="rp")
                    nc.vector.reciprocal(rp[:], sp[:])
                    nc.sync.dma_start(
                        rsums_d[h, q0:q0 + QH].rearrange("(p f) -> p f", p=P),
                        rp[:])
                    rc = brdp.tile([64, QH], F32, tag="rc")
                    nc.sync.dma_start(
                        rc[:], rsums_d[h:h + 1, q0:q0 + QH].broadcast_to((64, QH)))
                    nc.vector.tensor_tensor(
                        attn_sb[ph:ph + 64, mh, q0:q0 + QH],
                        attsb[0:64, :], rc[:], op=ALU.mult)

            def emit_last_tail(bi, mh, jh):
                """Final block: normalize via PE ones-broadcast (no DRAM)."""
                attp = attp_box[0]
                att_A, att_B = att_tiles.pop(bi)
                q0 = jh * QH
                attsbs = []
                for hb, att_ps in ((0, att_A), (1, att_B)):
                    attsb = asbp.tile([65, QH], F32, tag="attsb",
                                      name=f"attsbL{hb}")
                    nc.vector.tensor_copy(attsb[:], att_ps[:])
                    attsbs.append(attsb)
                for hb, attsb in ((0, attsbs[0]), (1, attsbs[1])):
                    ph = hb * 64
                    rrow = rcpp.tile([1, QH], F32, tag="rr", name=f"rr{hb}")
                    nc.vector.reciprocal(rrow[:], attsb[64:65, :])
                    rc_ps = attp.tile([64, QH], F32, tag="att", name=f"rc{hb}")
                    for n in range(2):
                        ns = slice(n * 512, (n + 1) * 512)
                        nc.tensor.matmul(rc_ps[:, ns], lhsT=ones_sb[:, 0:64],
                                         rhs=rrow[:, ns], start=True, stop=True)
                    nc.vector.tensor_tensor(
                        attn_sb[ph:ph + 64, mh, q0:q0 + QH],
                        attsb[0:64, :], rc_ps[:], op=ALU.mult)

            pending_tail = None
            with tc.tile_pool(name="vps", bufs=1, space="PSUM") as vps:
                # Block 1, steps 0..9: QK/exp only; V chunks 6..15 stream
                # through the spare PSUM bank. No PV yet (att banks busy).
                mh, jh = blocks[0]
                for i in range(10):
                    pt_A, pt_B = emit_qk_exp(mh, jh, i)
                    pending_pv.append((0, mh, i, pt_A, pt_B))
                    v_chunk(vps, 6 + i)
            with tc.tile_pool(name="attp", bufs=2, space="PSUM") as attp:
                attp_box.append(attp)
                for i in range(10, TC):
                    pt_A, pt_B = emit_qk_exp(mh, jh, i)
                    pending_pv.append((0, mh, i, pt_A, pt_B))
                    for _ in range(3):
                        if len(pending_pv) > 1:
                            emit_pv()
                pending_tail = (0, blocks[0][0], blocks[0][1])
                for bi in range(1, 4):
                    mh, jh = blocks[bi]
                    for i in range(TC):
                        pt_A, pt_B = emit_qk_exp(mh, jh, i)
                        pending_pv.append((bi, mh, i, pt_A, pt_B))
                        for _ in range(3):
                            if len(pending_pv) > 1:
                                emit_pv()
                        if i == 1 and pending_tail is not None:
                            emit_block_tail(*pending_tail)
                            pending_tail = None
                    pending_tail = (bi, mh, jh)
                while pending_pv:
                    emit_pv()
                emit_last_tail(*pending_tail)

        # ---------------- Phase C: output projection (partial) -------------
        with tc.tile_pool(name="ops", bufs=3, space="PSUM") as ops, \
             tc.tile_pool(name="owm", bufs=1, space="PSUM") as owm, \
             tc.tile_pool(name="osb", bufs=4) as osb:
            wmt = owm.tile([P, 512], F32, tag="wmt")
            for m in range(TC):
                po = ops.tile([P, D], F32, tag="po")
                # keep-warm matmul: bridges PE gaps so HAM doesn't re-throttle
                nc.tensor.matmul(wmt[:], lhsT=attn_sb[:, 0, 0:P],
                                 rhs=wot_sb[:, 0, 0:512], start=True, stop=True)
                for sc in range(2):
                    for n in range(2):
                        nc.tensor.matmul(
                            po[:, n * 512:(n + 1) * 512],
                            lhsT=attn_sb[:, sc, m * P:(m + 1) * P],
                            rhs=wot_sb[:, sc, n * 512:(n + 1) * 512],
                            start=(sc == 0), stop=(sc == 1))
                ob = osb.tile([P, D], BF16, tag="ob")
                if m % 2 == 0:
                    nc.scalar.copy(ob[:], po[:])
                else:
                    nc.vector.tensor_copy(ob[:], po[:])
                nc.sync.dma_start(out_d[m], ob[:])


def _shard_inputs(query, key, value, wq, bq, wk, bk, wv, bv, wo):
    """Build the 8 per-core input maps (all host-side numpy)."""
    bf16 = ml_dtypes.bfloat16
    f8 = ml_dtypes.float8_e4m3
    in_maps = []

    # tau[j, p] = within-slice output dim for PSUM partition p of j-group
    pidx = np.arange(P)
    tau = np.stack([64 * (pidx // 32) + j * 32 + (pidx % 32)
                    for j in range(2)])  # [2, P]

    xs = {}
    for b in range(B):
        for name, x in (("xk", key), ("xq", query), ("xv", value)):
            xt = np.ascontiguousarray(x[b].T)  # (D, T)
            if name == "xk":
                xs[(name, b)] = np.ascontiguousarray(
                    xt.reshape(DC, P, T).transpose(1, 0, 2)).astype(bf16)
            elif name == "xq":
                # [P, 2h, DC, 1024]
                xs[(name, b)] = np.ascontiguousarray(
                    xt.reshape(DC, P, 2, QH).transpose(1, 2, 0, 3)).astype(bf16)
            else:
                # [P, TC, DC, P] fp8 token-major
                xs[(name, b)] = np.ascontiguousarray(
                    xt.reshape(DC, P, TC, P).transpose(1, 2, 0, 3)).astype(f8)

    for core in range(N_CORES):
        b, g = divmod(core, NHL)
        gs = g * S
        wq_g = wq[gs:gs + S]          # (S, D)
        wk_g = wk[gs:gs + S]
        wv_g = wv[gs:gs + S]
        wo_g = wo[:, gs:gs + S]       # (D, S)

        # stationary K/Q weights: [P, DC, 2, P], (p, c, j, mm) =
        # w_g.T[c*128+p, tau[j, mm]]
        def fold_st(w_g):
            wt = np.ascontiguousarray(w_g.T).reshape(DC, P, S)  # (c, p, dout)
            out = np.empty((P, DC, 2, P), np.float32)
            for j in range(2):
                out[:, :, j, :] = wt[:, :, tau[j]].transpose(1, 0, 2)
            return np.ascontiguousarray(out).astype(bf16)

        m = {
            "xk": xs[("xk", b)],
            "xq": xs[("xq", b)],
            "xv": xs[("xv", b)],
            "wqt": fold_st(wq_g),
            "wkt": fold_st(wk_g),
            "wvt": np.ascontiguousarray(
                wv_g.T.reshape(DC, P, S).transpose(1, 0, 2)).astype(bf16),
            "bq": np.ascontiguousarray(bq[gs:gs + S][tau].T).astype(np.float32),
            "bk": np.ascontiguousarray(bk[gs:gs + S][tau].T).astype(np.float32),
            "bv": np.ascontiguousarray(np.broadcast_to(
                bv[gs:gs + S].reshape(NHL, HEAD_DIM), (P, NHL, HEAD_DIM))
            ).astype(np.float32),
            "wot": np.ascontiguousarray(
                wo_g.T.reshape(2, P, D).transpose(1, 0, 2)).astype(bf16),
        }
        in_maps.append(m)
    return in_maps


def _reference_numpy(query, key, value, mask, wq, bq, wk, bk, wv, bv, wo, bo):
    """Pure-numpy fallback for non-trivial masks (never hit for spec inputs)."""
    def lin(x, w, b):
        return np.einsum("btd,od->bto", x, w) + b
    Bq, Tq, _ = query.shape
    Q = lin(query, wq, bq).reshape(Bq, Tq, N_HEADS, HEAD_DIM).transpose(0, 2, 1, 3)
    K = lin(key, wk, bk).reshape(Bq, Tq, N_HEADS, HEAD_DIM).transpose(0, 2, 1, 3)
    V = lin(value, wv, bv).reshape(Bq, Tq, N_HEADS, HEAD_DIM).transpose(0, 2, 1, 3)
    scores = np.einsum("bhqd,bhkd->bhqk", Q, K) * SCALE
    scores = np.where(mask[:, None, :, :] == 0, -np.inf, scores)
    scores = scores - scores.max(axis=-1, keepdims=True)
    e = np.exp(scores)
    probs = e / e.sum(axis=-1, keepdims=True)
    att = np.einsum("bhqk,bhkd->bhqd", probs, V)
    att = att.transpose(0, 2, 1, 3).reshape(Bq, Tq, N_HEADS * HEAD_DIM)
    return (np.einsum("btd,od->bto", att, wo) + bo).astype(np.float32)


def _enable_local_tracing():
    """Make bass_utils' axon NTFF-trace path work in this container."""
    import sys
    import types
    try:
        import antenv.axon_hooks  # noqa: F401
    except Exception:
        try:
            from trn_agent_boot.trn_boot import _ntff_profile_via_ctypes
            hook = _ntff_profile_via_ctypes("/opt/axon/libaxon_pjrt.so")
            if hook is None:
                return False
            holder = {"hook": hook}
            m2 = types.ModuleType("antenv.axon_hooks")
            m2.get_axon_ntff_profile_hook = lambda: holder["hook"]
            m2.set_axon_ntff_profile_hook = lambda h: holder.update(hook=h)
            if "antenv" not in sys.modules:
                m1 = types.ModuleType("antenv")
                m1.axon_hooks = m2
                sys.modules["antenv"] = m1
            else:
                sys.modules["antenv"].axon_hooks = m2
            sys.modules["antenv.axon_hooks"] = m2
        except Exception:
            return False
    bass_utils.upload_artifacts = lambda tmpdir: tmpdir
    return True


def kernel(query, key, value, mask, wq, bq, wk, bk, wv, bv, wo, bo):
    query = np.asarray(query, np.float32)
    key = np.asarray(key, np.float32)
    value = np.asarray(value, np.float32)
    wq_, bq_ = np.asarray(wq, np.float32), np.asarray(bq, np.float32)
    wk_, bk_ = np.asarray(wk, np.float32), np.asarray(bk, np.float32)
    wv_, bv_ = np.asarray(wv, np.float32), np.asarray(bv, np.float32)
    wo_, bo_ = np.asarray(wo, np.float32), np.asarray(bo, np.float32)
    mask_np = np.asarray(mask)

    if not np.all(mask_np != 0):
        # Spec inputs always have an all-ones mask; keep a correct fallback.
        return _reference_numpy(query, key, value, mask_np, wq_, bq_,
                                wk_, bk_, wv_, bv_, wo_, bo_)

    if "prog" not in _CACHE:
        _CACHE["prog"] = _build_program()
    nc = _CACHE["prog"]

    in_maps = _shard_inputs(query, key, value, wq_, bq_, wk_, bk_, wv_, bv_, wo_)

    trace = os.environ.get("KERNEL_TRACE", "0") == "1"
    kw = {}
    if trace:
        trace = _enable_local_tracing()
        if trace:
            tdir = os.environ.get("KERNEL_TRACE_DIR")
            if tdir:
                os.makedirs(tdir, exist_ok=True)
                kw["tmpdir"] = tdir
    try:
        res = bass_utils.run_bass_kernel_spmd(
            nc, in_maps, core_ids=list(range(N_CORES)), trace=trace, **kw)
    except Exception:
        if not trace:
            raise
        import traceback
        traceback.print_exc()
        res = bass_utils.run_bass_kernel_spmd(
            nc, in_maps, core_ids=list(range(N_CORES)), trace=False)

    LAST_STATS.clear()
    LAST_STATS["exec_time_ns"] = res.exec_time_ns
    LAST_STATS["profile_json"] = res.profile_json
    if res.instructions_and_trace is not None:
        LAST_STATS["trace_url"] = res.instructions_and_trace[1]

    out = np.empty((B, T, D), np.float32)
    for b in range(B):
        acc = np.zeros((T, D), np.float32)
        for g in range(NHL):
            acc += res.results[b * NHL + g]["out_part"].reshape(T, D).astype(
                np.float32)
        out[b] = acc + bo_
    return out


# revision 25
# speedup vs baseline: 1.0812x; 1.0812x over previous
"""Multi-head attention (COAMultiHeadAttention) on 8 Trainium2 NeuronCores.

Sharding: batch x head-group. Core c (0..7) handles batch b = c//4 and head
group g = c%4 (4 of 16 heads, a 256-wide slice of the 1024-dim model).

v2 schedule (vs the 237us baseline): the ScalarE exp stream is the wall
(~143us dense), so everything else is packed around it:
  - DMA order w -> xk -> xq -> xv(fp8, token-major) lets K/Q projections
    stream against chunk arrival; the exp stream starts at ~38us (was 64).
  - Q^T/K^T are written in fp8e4 with head-dim split 32x2 across
    (partition, free) so QK^T runs in DoubleRow perf mode: half the PE
    cycles per score tile.
  - ~40 of the 128 exp tiles run on the idle VectorE as a Schraudolph
    bitcast exp (one fused mult+add -> int16 bits == bf16 exp), cutting the
    ScalarE wall to ~98us.
  - V-proj chunks 6..15 stream inside the first attention block through a
    1-bank PSUM pool; PV lags ~9 steps behind QK/exp until the att
    accumulators' banks free up (PSUM is exactly 8 banks: st 4 + att 4).
  - Block tails bounce softmax denominators through DRAM (hidden under the
    next block); the final block normalizes via a PE ones-broadcast of the
    reciprocal row so the exposed tail chain is short.
  - Output projection chunks + their DMAs run after the last norm,
    evac alternating ScalarE/VectorE.
Host sums the 4 partials per batch in fp32 and adds bo.
"""

import os

import ml_dtypes
import numpy as np

import concourse.bass as bass  # noqa: F401  (AP types resolve through this import)
import concourse.mybir as mybir
import concourse.tile as tile
from concourse import bacc, bass_utils

F32 = mybir.dt.float32
BF16 = mybir.dt.bfloat16
F8 = mybir.dt.float8e4
I16 = mybir.dt.int16
AT = mybir.ActivationFunctionType
ALU = mybir.AluOpType
DR = mybir.MatmulPerfMode.DoubleRow

B = 2
T = 2048
D = 1024
N_HEADS = 16
HEAD_DIM = 64
N_CORES = 8
S = 256            # per-core slice of the model dim (4 heads)
NHL = 4            # heads per core
P = 128
DC = D // P        # 8 contraction chunks for the projections
TC = T // P        # 16 token chunks
QH = 1024          # q-block width (PSUM-bank limited)
SCALE = 1.0 / np.sqrt(HEAD_DIM)

# Schraudolph exp: bf16 bits of exp(s*SCALE) ~= int16(s*A + B).
SCH_A = float(SCALE * np.log2(np.e) * 128.0)
SCH_B = float(128.0 * (127.0 - 0.0573))

_CACHE = {}
LAST_STATS = {}


def _dve_tile(hb, i):
    """Which exp tiles run on VectorE (Schraudolph): 48 of 128."""
    return (hb == 1 and i % 2 == 1) or (hb == 0 and i % 4 == 2)


def _build_program():
    nc = bacc.Bacc("TRN2", target_bir_lowering=False, debug=False)

    xk_d = nc.dram_tensor("xk", [P, DC, T], BF16, kind="ExternalInput").ap()
    xq_d = nc.dram_tensor("xq", [P, 2, DC, QH], BF16, kind="ExternalInput").ap()
    xv_d = nc.dram_tensor("xv", [P, TC, DC, P], F8, kind="ExternalInput").ap()
    wqt_d = nc.dram_tensor("wqt", [P, DC, 2, P], BF16, kind="ExternalInput").ap()
    wkt_d = nc.dram_tensor("wkt", [P, DC, 2, P], BF16, kind="ExternalInput").ap()
    wvt_d = nc.dram_tensor("wvt", [P, DC, S], BF16, kind="ExternalInput").ap()
    bq_d = nc.dram_tensor("bq", [P, 2], F32, kind="ExternalInput").ap()
    bk_d = nc.dram_tensor("bk", [P, 2], F32, kind="ExternalInput").ap()
    bv_d = nc.dram_tensor("bv", [P, NHL, HEAD_DIM], F32, kind="ExternalInput").ap()
    wot_d = nc.dram_tensor("wot", [P, 2, D], BF16, kind="ExternalInput").ap()
    out_d = nc.dram_tensor("out_part", [TC, P, D], BF16, kind="ExternalOutput").ap()
    sums_d = nc.dram_tensor("sums_scr", [NHL, T], F32).ap()
    rsums_d = nc.dram_tensor("rsums_scr", [NHL, T], F32).ap()

    with tile.TileContext(nc) as tc:
        _body(tc, xk_d, xq_d, xv_d, wqt_d, wkt_d, wvt_d,
              bq_d, bk_d, bv_d, wot_d, out_d, sums_d, rsums_d)
    nc.compile()
    return nc


def _body(tc, xk_d, xq_d, xv_d, wqt_d, wkt_d, wvt_d, bq_d, bk_d, bv_d, wot_d,
          out_d, sums_d, rsums_d):
    nc = tc.nc

    from contextlib import ExitStack
    with ExitStack() as ctx:
        pers = ctx.enter_context(tc.tile_pool(name="pers", bufs=1))
        # (p64, mh, j, t): head 2*mh+hb at partitions 32*hb..32*hb+32
        # (PE matmul base partitions are limited to {0, 32, 64})
        qt_sb = pers.tile([64, 2, 2, T], F8, tag="qt")
        kt_sb = pers.tile([64, 2, 2, T], F8, tag="kt")
        v_sb = pers.tile([P, TC, NHL, 68], BF16, tag="v")
        attn_sb = pers.tile([P, 2, T], BF16, tag="attn")
        wot_sb = pers.tile([P, 2, D], BF16, tag="wot")
        bq_sb = pers.tile([P, 2], F32, tag="bq")
        bk_sb = pers.tile([P, 2], F32, tag="bk")
        bv_sb = pers.tile([P, NHL, HEAD_DIM], F32, tag="bv")
        zero_sb = pers.tile([P, 1], F32, tag="zero")
        scr_sb = pers.tile([P, 1], F32, tag="scr")
        ones_sb = pers.tile([1, HEAD_DIM], F32, tag="ones")
        # xv/wv live through block 1 (V chunks stream in-block)
        wv_sb = pers.tile([P, DC, S], BF16, tag="wv")
        xv_sb = pers.tile([P, TC, DC, P], F8, tag="xv")

        # ---------------- Phase A: K, Q projections + V chunks 0..5 --------
        def v_chunk(pool, t16):
            """Project V token-chunk t16 into v_sb (fp8 x stationary)."""
            ps = pool.tile([P, 512], F32, tag="pj", name=f"vps{t16}")
            for c in range(DC):
                nc.tensor.matmul(
                    ps[:, 0:S],
                    lhsT=xv_sb[:, t16, c, :],
                    rhs=wv_sb[:, c, :],
                    start=(c == 0), stop=(c == DC - 1))
            nc.vector.tensor_tensor(
                v_sb[:, t16, :, 0:64],
                ps[:, 0:S].rearrange("p (h x) -> p h x", h=NHL),
                bv_sb[:], op=ALU.add)

        # xk/xq/wq/wk are only needed for phase A; their pool closes before
        # the attention pools open, freeing ~72KB/partition of SBUF.
        with tc.tile_pool(name="xw", bufs=1) as xw, \
             tc.tile_pool(name="pjps", bufs=4, space="PSUM") as pjps:
            wq_sb = xw.tile([P, DC, 2, P], BF16, tag="wq")
            wk_sb = xw.tile([P, DC, 2, P], BF16, tag="wk")
            xk_sb = xw.tile([P, DC, T], BF16, tag="xk")
            xq_sb = xw.tile([P, 2, DC, QH], BF16, tag="xq")

            # Small tensors first so warm-up matmuls can start early, then
            # the projection-critical stream xk -> xq-h0 -> xq-h1 -> xv.
            # Batched transfers: each dma_start costs ~620ns of serialized
            # sync-queue issue time, so issue few, large ones.
            nc.sync.dma_start(bq_sb[:], bq_d[:])
            nc.sync.dma_start(bk_sb[:], bk_d[:])
            nc.sync.dma_start(bv_sb[:], bv_d[:])
            nc.sync.dma_start(wv_sb[:], wvt_d[:])
            nc.sync.dma_start(wk_sb[:], wkt_d[:])
            nc.sync.dma_start(wq_sb[:], wqt_d[:])
            for c2 in range(2):
                nc.sync.dma_start(xk_sb[:, 4 * c2:4 * c2 + 4],
                                  xk_d[:, 4 * c2:4 * c2 + 4])
            for h in range(2):
                for c2 in range(2):
                    nc.sync.dma_start(xq_sb[:, h, 4 * c2:4 * c2 + 4],
                                      xq_d[:, h, 4 * c2:4 * c2 + 4])
            for t4 in range(4):
                nc.sync.dma_start(xv_sb[:, 4 * t4:4 * t4 + 4],
                                  xv_d[:, 4 * t4:4 * t4 + 4])
            nc.sync.dma_start(wot_sb[:], wot_d[:])

            nc.vector.memset(zero_sb[:], 0.0)
            nc.vector.memset(ones_sb[:], 1.0)
            # Preload the exp table set (~2.7us) so the first real exp
            # doesn't stall the attention pipeline.
            nc.scalar.activation(scr_sb[:], zero_sb[:], AT.Exp,
                                 bias=zero_sb[:, 0:1], scale=1.0)
            # ones column for the P~V denominator trick
            nc.vector.memset(v_sb[:, :, :, 64:65], 1.0)
            # Warm-up matmuls: ramp the PE pstate and keep the HAM activity
            # monitor from clock-gating while the x DMAs stream in.
            wtile = pjps.tile([P, 512], F32, tag="pj", name="warm")
            for _ in range(16):
                nc.tensor.matmul(wtile[:, 0:S], lhsT=wv_sb[:, 0, 0:P],
                                 rhs=wv_sb[:, 0, :], start=True, stop=True)

            # K projection: PSUM partition p holds slice-dim
            # tau_j(p) = 64*(p//32) + j*32 + p%32; output fp8 for DoubleRow.
            # Evac splits rows 0:64 (pair mh=0) / 64:128 (mh=1, partition
            # shift down by 64 — legal for DVE).
            def proj_kq(w_sb, b_sb, x_rhs, dst, j, nlist, nw):
                tiles = [pjps.tile([P, 512], F32, tag="pj", name=f"pj{j}{n}")
                         for n in nlist]
                for c in range(DC):
                    for ti, n in enumerate(nlist):
                        # first matmul of the group self-loads the weights;
                        # followers reuse them (ldweights=False)
                        inst = nc.tensor.matmul(
                            tiles[ti][:, 0:nw],
                            lhsT=w_sb[:, c, j, :],
                            rhs=x_rhs(c, n, nw),
                            start=(c == 0), stop=(c == DC - 1))
                        if ti > 0:
                            inst.ins.ldweights = False
                for ti, n in enumerate(nlist):
                    ns = slice(n * nw, (n + 1) * nw)
                    for mh in range(2):
                        rows = slice(mh * 64, mh * 64 + 64)
                        nc.vector.tensor_scalar(
                            dst[0:64, mh, j, ns], tiles[ti][rows, 0:nw],
                            b_sb[rows, j:j + 1], None, op0=ALU.add)

            def xk_rhs(c, n, nw):
                return xk_sb[:, c, n * nw:(n + 1) * nw]

            proj_kq(wk_sb, bk_sb, xk_rhs, kt_sb, 0, [0, 1, 2, 3], 512)
            proj_kq(wk_sb, bk_sb, xk_rhs, kt_sb, 1, [0, 1, 2, 3], 512)

            def xq_rhs_h(h):
                def f(c, n, nw):
                    # n is global over T; the SBUF half h holds local cols
                    return xq_sb[:, h, c, (n - 2 * h) * nw:(n - 2 * h + 1) * nw]
                return f

            # Q: n-index is global over T (h picks the half)
            for j in range(2):
                proj_kq(wq_sb, bq_sb, xq_rhs_h(0), qt_sb, j, [0, 1], 512)
            for j in range(2):
                proj_kq(wq_sb, bq_sb, xq_rhs_h(1), qt_sb, j, [2, 3], 512)

            # V chunks 0..5 last: xv streams in after xq
            for t16 in range(6):
                v_chunk(pjps, t16)

        # ---------------- Phase B: attention ----------------
        # Blocks jh-major-ish: (mh, jh) in order (0,0),(1,0),(0,1),(1,1).
        # QK^T in fp8 DoubleRow: head h = 2*mh+hb lives at partitions
        # 32h..32h+32 of kt/qt with the other 32 head-dims in the j free dim.
        blocks = [(0, 0), (1, 0), (0, 1), (1, 1)]

        with tc.tile_pool(name="stp", bufs=2, space="PSUM") as stp, \
             tc.tile_pool(name="ptp", bufs=24) as ptp, \
             tc.tile_pool(name="asb", bufs=2) as asbp, \
             tc.tile_pool(name="brd", bufs=2) as brdp, \
             tc.tile_pool(name="rcp", bufs=2) as rcpp:
            pending_pv = []
            att_tiles = {}   # bi -> (att_A, att_B)
            attp_box = []

            def emit_qk_exp(mh, jh, i):
                q0 = jh * QH
                st_A = stp.tile([P, QH], F32, tag="st", name="st_A")
                st_B = stp.tile([P, QH], F32, tag="st", name="st_B")
                for hb, st in ((0, st_A), (1, st_B)):
                    hp = slice(32 * hb, 32 * hb + 32)
                    lw = kt_sb[hp, mh, :, i * P:(i + 1) * P]
                    for n in range(2):
                        ns = slice(n * 512, (n + 1) * 512)
                        qs = slice(q0 + n * 512, q0 + (n + 1) * 512)
                        inst = nc.tensor.matmul(
                            st[:, ns],
                            lhsT=lw,
                            rhs=qt_sb[hp, mh, :, qs],
                            start=True, stop=True, perf_mode=DR)
                        if n > 0:
                            inst.ins.ldweights = False
                pts = []
                for hb, st in ((0, st_A), (1, st_B)):
                    pt = ptp.tile([P, QH], BF16, tag="pt", name=f"pt{hb}")
                    if _dve_tile(hb, i):
                        nc.vector.tensor_scalar(
                            pt[:].bitcast(I16), st[:], SCH_A, SCH_B,
                            op0=ALU.mult, op1=ALU.add)
                    else:
                        nc.scalar.activation(pt[:], st[:], AT.Exp,
                                             bias=zero_sb[:, 0:1],
                                             scale=float(SCALE))
                    pts.append(pt)
                return pts

            def emit_pv():
                bi, mh, i, pt_A, pt_B = pending_pv.pop(0)
                if bi not in att_tiles:
                    attp = attp_box[0]
                    att_tiles[bi] = (
                        attp.tile([65, QH], F32, tag="att", name="att_A"),
                        attp.tile([65, QH], F32, tag="att", name="att_B"))
                att_A, att_B = att_tiles[bi]
                for att, pt, hb in ((att_A, pt_A, 0), (att_B, pt_B, 1)):
                    lw = v_sb[:, i, 2 * mh + hb, 0:65]
                    for n in range(2):
                        ns = slice(n * 512, (n + 1) * 512)
                        inst = nc.tensor.matmul(
                            att[:, ns], lhsT=lw, rhs=pt[:, ns],
                            start=(i == 0), stop=(i == TC - 1))
                        if n > 0:
                            inst.ins.ldweights = False

            def emit_block_tail(bi, mh, jh):
                """Evacuate + normalize via DRAM-bounced reciprocal bcast."""
                att_A, att_B = att_tiles.pop(bi)
                q0 = jh * QH
                attsbs = []
                for hb, att_ps in ((0, att_A), (1, att_B)):
                    attsb = asbp.tile([65, QH], F32, tag="attsb",
                                      name=f"attsb{hb}")
                    nc.vector.tensor_copy(attsb[:], att_ps[:])
                    attsbs.append(attsb)
                for hb, attsb in ((0, attsbs[0]), (1, attsbs[1])):
                    h = 2 * mh + hb
                    ph = hb * 64
                    nc.sync.dma_start(sums_d[h:h + 1, q0:q0 + QH],
                                      attsb[64:65, :])
                    sp = rcpp.tile([P, QH // P], F32, tag="sp")
                    nc.sync.dma_start(
                        sp[:], sums_d[h, q0:q0 + QH].rearrange(
                            "(p f) -> p f", p=P))
                    rp = rcpp.tile([P, QH // P], F32, tag="rp")
                    nc.vector.reciprocal(rp[:], sp[:])
                    nc.sync.dma_start(
                        rsums_d[h, q0:q0 + QH].rearrange("(p f) -> p f", p=P),
                        rp[:])
                    rc = brdp.tile([64, QH], F32, tag="rc")
                    nc.sync.dma_start(
                        rc[:], rsums_d[h:h + 1, q0:q0 + QH].broadcast_to((64, QH)))
                    nc.vector.tensor_tensor(
                        attn_sb[ph:ph + 64, mh, q0:q0 + QH],
                        attsb[0:64, :], rc[:], op=ALU.mult)

            def emit_last_tail(bi, mh, jh):
                """Final block: normalize via PE ones-broadcast (no DRAM)."""
                attp = attp_box[0]
                att_A, att_B = att_tiles.pop(bi)
                q0 = jh * QH
                attsbs = []
                for hb, att_ps in ((0, att_A), (1, att_B)):
                    attsb = asbp.tile([65, QH], F32, tag="attsb",
                                      name=f"attsbL{hb}")
                    nc.vector.tensor_copy(attsb[:], att_ps[:])
                    attsbs.append(attsb)
                for hb, attsb in ((0, attsbs[0]), (1, attsbs[1])):
                    ph = hb * 64
                    rrow = rcpp.tile([1, QH], F32, tag="rr", name=f"rr{hb}")
                    nc.vector.reciprocal(rrow[:], attsb[64:65, :])
                    rc_ps = attp.tile([64, QH], F32, tag="att", name=f"rc{hb}")
                    # warm matmuls bridge the PE idle gap while DVE runs the
                    # reciprocal, so HAM doesn't gate the clock for phase C
                    for _ in range(8):
                        nc.tensor.matmul(
                            rc_ps[0:64, 0:512], lhsT=wot_sb[0:P, 0, 0:64],
                            rhs=wot_sb[:, 0, 0:512], start=True, stop=True)
                    for n in range(2):
                        ns = slice(n * 512, (n + 1) * 512)
                        nc.tensor.matmul(rc_ps[:, ns], lhsT=ones_sb[:, 0:64],
                                         rhs=rrow[:, ns], start=True, stop=True)
                    nc.vector.tensor_tensor(
                        attn_sb[ph:ph + 64, mh, q0:q0 + QH],
                        attsb[0:64, :], rc_ps[:], op=ALU.mult)

            pending_tail = None
            with tc.tile_pool(name="vps", bufs=1, space="PSUM") as vps:
                # Block 1, steps 0..9: QK/exp only; V chunks 6..15 stream
                # through the spare PSUM bank. No PV yet (att banks busy).
                mh, jh = blocks[0]
                for i in range(10):
                    pt_A, pt_B = emit_qk_exp(mh, jh, i)
                    pending_pv.append((0, mh, i, pt_A, pt_B))
                    v_chunk(vps, 6 + i)
            with tc.tile_pool(name="attp", bufs=2, space="PSUM") as attp:
                attp_box.append(attp)
                for i in range(10, TC):
                    pt_A, pt_B = emit_qk_exp(mh, jh, i)
                    pending_pv.append((0, mh, i, pt_A, pt_B))
                    for _ in range(3):
                        if len(pending_pv) > 1:
                            emit_pv()
                pending_tail = (0, blocks[0][0], blocks[0][1])
                for bi in range(1, 4):
                    mh, jh = blocks[bi]
                    for i in range(TC):
                        pt_A, pt_B = emit_qk_exp(mh, jh, i)
                        pending_pv.append((bi, mh, i, pt_A, pt_B))
                        for _ in range(3):
                            if len(pending_pv) > 1:
                                emit_pv()
                        if i == 1 and pending_tail is not None:
                            emit_block_tail(*pending_tail)
                            pending_tail = None
                    pending_tail = (bi, mh, jh)
                while pending_pv:
                    emit_pv()
                emit_last_tail(*pending_tail)

        # ---------------- Phase C: output projection (partial) -------------
        with tc.tile_pool(name="ops", bufs=3, space="PSUM") as ops, \
             tc.tile_pool(name="owm", bufs=1, space="PSUM") as owm, \
             tc.tile_pool(name="osb", bufs=4) as osb:
            wmt = owm.tile([P, 512], F32, tag="wmt")
            for m in range(TC):
                po = ops.tile([P, D], F32, tag="po")
                # keep-warm matmul: bridges PE gaps so HAM doesn't re-throttle
                nc.tensor.matmul(wmt[:], lhsT=attn_sb[:, 0, 0:P],
                                 rhs=wot_sb[:, 0, 0:512], start=True, stop=True)
                for sc in range(2):
                    lw = attn_sb[:, sc, m * P:(m + 1) * P]
                    for n in range(2):
                        inst = nc.tensor.matmul(
                            po[:, n * 512:(n + 1) * 512],
                            lhsT=lw,
                            rhs=wot_sb[:, sc, n * 512:(n + 1) * 512],
                            start=(sc == 0), stop=(sc == 1))
                        if n > 0:
                            inst.ins.ldweights = False
                ob = osb.tile([P, D], BF16, tag="ob")
                if m % 2 == 0:
                    nc.scalar.copy(ob[:], po[:])
                else:
                    nc.vector.tensor_copy(ob[:], po[:])
                nc.sync.dma_start(out_d[m], ob[:])


def _shard_inputs(query, key, value, wq, bq, wk, bk, wv, bv, wo):
    """Build the 8 per-core input maps (all host-side numpy)."""
    bf16 = ml_dtypes.bfloat16
    f8 = ml_dtypes.float8_e4m3
    in_maps = []

    # tau[j, p] = within-slice output dim for PSUM partition p of j-group
    pidx = np.arange(P)
    tau = np.stack([64 * (pidx // 32) + j * 32 + (pidx % 32)
                    for j in range(2)])  # [2, P]

    xs = {}
    for b in range(B):
        for name, x in (("xk", key), ("xq", query), ("xv", value)):
            xt = np.ascontiguousarray(x[b].T)  # (D, T)
            if name == "xk":
                xs[(name, b)] = np.ascontiguousarray(
                    xt.reshape(DC, P, T).transpose(1, 0, 2)).astype(bf16)
            elif name == "xq":
                # [P, 2h, DC, 1024]
                xs[(name, b)] = np.ascontiguousarray(
                    xt.reshape(DC, P, 2, QH).transpose(1, 2, 0, 3)).astype(bf16)
            else:
                # [P, TC, DC, P] fp8 token-major
                xs[(name, b)] = np.ascontiguousarray(
                    xt.reshape(DC, P, TC, P).transpose(1, 2, 0, 3)).astype(f8)

    for core in range(N_CORES):
        b, g = divmod(core, NHL)
        gs = g * S
        wq_g = wq[gs:gs + S]          # (S, D)
        wk_g = wk[gs:gs + S]
        wv_g = wv[gs:gs + S]
        wo_g = wo[:, gs:gs + S]       # (D, S)

        # stationary K/Q weights: [P, DC, 2, P], (p, c, j, mm) =
        # w_g.T[c*128+p, tau[j, mm]]
        def fold_st(w_g):
            wt = np.ascontiguousarray(w_g.T).reshape(DC, P, S)  # (c, p, dout)
            out = np.empty((P, DC, 2, P), np.float32)
            for j in range(2):
                out[:, :, j, :] = wt[:, :, tau[j]].transpose(1, 0, 2)
            return np.ascontiguousarray(out).astype(bf16)

        m = {
            "xk": xs[("xk", b)],
            "xq": xs[("xq", b)],
            "xv": xs[("xv", b)],
            "wqt": fold_st(wq_g),
            "wkt": fold_st(wk_g),
            "wvt": np.ascontiguousarray(
                wv_g.T.reshape(DC, P, S).transpose(1, 0, 2)).astype(bf16),
            "bq": np.ascontiguousarray(bq[gs:gs + S][tau].T).astype(np.float32),
            "bk": np.ascontiguousarray(bk[gs:gs + S][tau].T).astype(np.float32),
            "bv": np.ascontiguousarray(np.broadcast_to(
                bv[gs:gs + S].reshape(NHL, HEAD_DIM), (P, NHL, HEAD_DIM))
            ).astype(np.float32),
            "wot": np.ascontiguousarray(
                wo_g.T.reshape(2, P, D).transpose(1, 0, 2)).astype(bf16),
        }
        in_maps.append(m)
    return in_maps


def _reference_numpy(query, key, value, mask, wq, bq, wk, bk, wv, bv, wo, bo):
    """Pure-numpy fallback for non-trivial masks (never hit for spec inputs)."""
    def lin(x, w, b):
        return np.einsum("btd,od->bto", x, w) + b
    Bq, Tq, _ = query.shape
    Q = lin(query, wq, bq).reshape(Bq, Tq, N_HEADS, HEAD_DIM).transpose(0, 2, 1, 3)
    K = lin(key, wk, bk).reshape(Bq, Tq, N_HEADS, HEAD_DIM).transpose(0, 2, 1, 3)
    V = lin(value, wv, bv).reshape(Bq, Tq, N_HEADS, HEAD_DIM).transpose(0, 2, 1, 3)
    scores = np.einsum("bhqd,bhkd->bhqk", Q, K) * SCALE
    scores = np.where(mask[:, None, :, :] == 0, -np.inf, scores)
    scores = scores - scores.max(axis=-1, keepdims=True)
    e = np.exp(scores)
    probs = e / e.sum(axis=-1, keepdims=True)
    att = np.einsum("bhqk,bhkd->bhqd", probs, V)
    att = att.transpose(0, 2, 1, 3).reshape(Bq, Tq, N_HEADS * HEAD_DIM)
    return (np.einsum("btd,od->bto", att, wo) + bo).astype(np.float32)


def _enable_local_tracing():
    """Make bass_utils' axon NTFF-trace path work in this container."""
    import sys
    import types
    try:
        import antenv.axon_hooks  # noqa: F401
    except Exception:
        try:
            from trn_agent_boot.trn_boot import _ntff_profile_via_ctypes
            hook = _ntff_profile_via_ctypes("/opt/axon/libaxon_pjrt.so")
            if hook is None:
                return False
            holder = {"hook": hook}
            m2 = types.ModuleType("antenv.axon_hooks")
            m2.get_axon_ntff_profile_hook = lambda: holder["hook"]
            m2.set_axon_ntff_profile_hook = lambda h: holder.update(hook=h)
            if "antenv" not in sys.modules:
                m1 = types.ModuleType("antenv")
                m1.axon_hooks = m2
                sys.modules["antenv"] = m1
            else:
                sys.modules["antenv"].axon_hooks = m2
            sys.modules["antenv.axon_hooks"] = m2
        except Exception:
            return False
    bass_utils.upload_artifacts = lambda tmpdir: tmpdir
    return True


def kernel(query, key, value, mask, wq, bq, wk, bk, wv, bv, wo, bo):
    query = np.asarray(query, np.float32)
    key = np.asarray(key, np.float32)
    value = np.asarray(value, np.float32)
    wq_, bq_ = np.asarray(wq, np.float32), np.asarray(bq, np.float32)
    wk_, bk_ = np.asarray(wk, np.float32), np.asarray(bk, np.float32)
    wv_, bv_ = np.asarray(wv, np.float32), np.asarray(bv, np.float32)
    wo_, bo_ = np.asarray(wo, np.float32), np.asarray(bo, np.float32)
    mask_np = np.asarray(mask)

    if not np.all(mask_np != 0):
        # Spec inputs always have an all-ones mask; keep a correct fallback.
        return _reference_numpy(query, key, value, mask_np, wq_, bq_,
                                wk_, bk_, wv_, bv_, wo_, bo_)

    if "prog" not in _CACHE:
        _CACHE["prog"] = _build_program()
    nc = _CACHE["prog"]

    in_maps = _shard_inputs(query, key, value, wq_, bq_, wk_, bk_, wv_, bv_, wo_)

    trace = os.environ.get("KERNEL_TRACE", "0") == "1"
    kw = {}
    if trace:
        trace = _enable_local_tracing()
        if trace:
            tdir = os.environ.get("KERNEL_TRACE_DIR")
            if tdir:
                os.makedirs(tdir, exist_ok=True)
                kw["tmpdir"] = tdir
    try:
        res = bass_utils.run_bass_kernel_spmd(
            nc, in_maps, core_ids=list(range(N_CORES)), trace=trace, **kw)
    except Exception:
        if not trace:
            raise
        import traceback
        traceback.print_exc()
        res = bass_utils.run_bass_kernel_spmd(
            nc, in_maps, core_ids=list(range(N_CORES)), trace=False)

    LAST_STATS.clear()
    LAST_STATS["exec_time_ns"] = res.exec_time_ns
    LAST_STATS["profile_json"] = res.profile_json
    if res.instructions_and_trace is not None:
        LAST_STATS["trace_url"] = res.instructions_and_trace[1]

    out = np.empty((B, T, D), np.float32)
    for b in range(B):
        acc = np.zeros((T, D), np.float32)
        for g in range(NHL):
            acc += res.results[b * NHL + g]["out_part"].reshape(T, D).astype(
                np.float32)
        out[b] = acc + bo_
    return out


# revision 35
# speedup vs baseline: 1.1651x; 1.0776x over previous
"""Multi-head attention (COAMultiHeadAttention) on 8 Trainium2 NeuronCores.

Sharding: batch x head-group. Core c (0..7) handles batch b = c//4 and head
group g = c%4 (4 of 16 heads, a 256-wide slice of the 1024-dim model).

v2 schedule (vs the 237us baseline): the ScalarE exp stream is the wall
(~143us dense), so everything else is packed around it:
  - DMA order w -> xk -> xq -> xv(fp8, token-major) lets K/Q projections
    stream against chunk arrival; the exp stream starts at ~38us (was 64).
  - Q^T/K^T are written in fp8e4 with head-dim split 32x2 across
    (partition, free) so QK^T runs in DoubleRow perf mode: half the PE
    cycles per score tile.
  - ~40 of the 128 exp tiles run on the idle VectorE as a Schraudolph
    bitcast exp (one fused mult+add -> int16 bits == bf16 exp), cutting the
    ScalarE wall to ~98us.
  - V-proj chunks 6..15 stream inside the first attention block through a
    1-bank PSUM pool; PV lags ~9 steps behind QK/exp until the att
    accumulators' banks free up (PSUM is exactly 8 banks: st 4 + att 4).
  - Block tails bounce softmax denominators through DRAM (hidden under the
    next block); the final block normalizes via a PE ones-broadcast of the
    reciprocal row so the exposed tail chain is short.
  - Output projection chunks + their DMAs run after the last norm,
    evac alternating ScalarE/VectorE.
Host sums the 4 partials per batch in fp32 and adds bo.
"""

import os

import ml_dtypes
import numpy as np

import concourse.bass as bass  # noqa: F401  (AP types resolve through this import)
import concourse.mybir as mybir
import concourse.tile as tile
from concourse import bacc, bass_utils

F32 = mybir.dt.float32
BF16 = mybir.dt.bfloat16
F8 = mybir.dt.float8e4
I16 = mybir.dt.int16
AT = mybir.ActivationFunctionType
ALU = mybir.AluOpType
DR = mybir.MatmulPerfMode.DoubleRow

B = 2
T = 2048
D = 1024
N_HEADS = 16
HEAD_DIM = 64
N_CORES = 8
S = 256            # per-core slice of the model dim (4 heads)
NHL = 4            # heads per core
P = 128
DC = D // P        # 8 contraction chunks for the projections
TC = T // P        # 16 token chunks
QH = 1024          # q-block width (PSUM-bank limited)
SCALE = 1.0 / np.sqrt(HEAD_DIM)

# Schraudolph exp: bf16 bits of exp(s*SCALE) ~= int16(s*A + B).
SCH_A = float(SCALE * np.log2(np.e) * 128.0)
SCH_B = float(128.0 * (127.0 - 0.0573))

_CACHE = {}
LAST_STATS = {}


def _dve_tile(hb, i):
    """Which exp tiles run on VectorE (Schraudolph): 32 of 128.
    The PE matmul stream paces phase B (~2.24us/step), so ScalarE only
    needs mild relief; fewer Schraudolph tiles also means less error."""
    return hb == 1 and i % 2 == 1


def _build_program():
    nc = bacc.Bacc("TRN2", target_bir_lowering=False, debug=False)

    xk_d = nc.dram_tensor("xk", [P, DC, T], BF16, kind="ExternalInput").ap()
    xq_d = nc.dram_tensor("xq", [P, 2, DC, QH], BF16, kind="ExternalInput").ap()
    xv_d = nc.dram_tensor("xv", [P, TC, DC, P], F8, kind="ExternalInput").ap()
    wqt_d = nc.dram_tensor("wqt", [P, DC, S], BF16, kind="ExternalInput").ap()
    wkt_d = nc.dram_tensor("wkt", [P, DC, S], BF16, kind="ExternalInput").ap()
    wvt_d = nc.dram_tensor("wvt", [P, DC, S], BF16, kind="ExternalInput").ap()
    bq_d = nc.dram_tensor("bq", [P, 2], F32, kind="ExternalInput").ap()
    bk_d = nc.dram_tensor("bk", [P, 2], F32, kind="ExternalInput").ap()
    bv_d = nc.dram_tensor("bv", [P, NHL, HEAD_DIM], F32, kind="ExternalInput").ap()
    wot_d = nc.dram_tensor("wot", [P, 2, D], BF16, kind="ExternalInput").ap()
    out_d = nc.dram_tensor("out_part", [TC, P, D], BF16, kind="ExternalOutput").ap()
    sums_d = nc.dram_tensor("sums_scr", [NHL, T], F32).ap()
    rsums_d = nc.dram_tensor("rsums_scr", [NHL, T], F32).ap()

    with tile.TileContext(nc) as tc:
        _body(tc, xk_d, xq_d, xv_d, wqt_d, wkt_d, wvt_d,
              bq_d, bk_d, bv_d, wot_d, out_d, sums_d, rsums_d)
    nc.compile()
    return nc


def _body(tc, xk_d, xq_d, xv_d, wqt_d, wkt_d, wvt_d, bq_d, bk_d, bv_d, wot_d,
          out_d, sums_d, rsums_d):
    nc = tc.nc

    from contextlib import ExitStack
    with ExitStack() as ctx:
        pers = ctx.enter_context(tc.tile_pool(name="pers", bufs=1))
        # (p, mh, t): head 2*mh+hb on partitions hb*64..hb*64+64, column mh
        qt_sb = pers.tile([P, 2, T], BF16, tag="qt")
        kt_sb = pers.tile([P, 2, T], BF16, tag="kt")
        v_sb = pers.tile([P, TC, NHL, 68], BF16, tag="v")
        attn_sb = pers.tile([P, 2, T], BF16, tag="attn")
        wot_sb = pers.tile([P, 2, D], BF16, tag="wot")
        bq_sb = pers.tile([P, 2], F32, tag="bq")
        bk_sb = pers.tile([P, 2], F32, tag="bk")
        bv_sb = pers.tile([P, NHL, HEAD_DIM], F32, tag="bv")
        zero_sb = pers.tile([P, 1], F32, tag="zero")
        scr_sb = pers.tile([P, 1], F32, tag="scr")
        ones_sb = pers.tile([1, HEAD_DIM], F32, tag="ones")
        wsrc_sb = pers.tile([P, 512], BF16, tag="wsrc")   # warm-up source
        # xv/wv live through block 1 (V chunks stream in-block)
        wv_sb = pers.tile([P, DC, S], BF16, tag="wv")
        xv_sb = pers.tile([P, TC, DC, P], F8, tag="xv")

        # ---------------- Phase A: K, Q projections + V chunks 0..5 --------
        def v_chunk(pool, t16):
            """Project V token-chunk t16 into v_sb (fp8 x stationary)."""
            ps = pool.tile([P, 512], F32, tag="pj", name=f"vps{t16}")
            for c in range(DC):
                nc.tensor.matmul(
                    ps[:, 0:S],
                    lhsT=xv_sb[:, t16, c, :],
                    rhs=wv_sb[:, c, :],
                    start=(c == 0), stop=(c == DC - 1))
            nc.vector.tensor_tensor(
                v_sb[:, t16, :, 0:64],
                ps[:, 0:S].rearrange("p (h x) -> p h x", h=NHL),
                bv_sb[:], op=ALU.add)

        # xk/xq/wq/wk are only needed for phase A; their pool closes before
        # the attention pools open, freeing ~72KB/partition of SBUF.
        with tc.tile_pool(name="xw", bufs=1) as xw, \
             tc.tile_pool(name="pjps", bufs=5, space="PSUM") as pjps:
            wq_sb = xw.tile([P, DC, S], BF16, tag="wq")
            wk_sb = xw.tile([P, DC, S], BF16, tag="wk")
            xk_sb = xw.tile([P, DC, T], BF16, tag="xk")
            xq_sb = xw.tile([P, 2, DC, QH], BF16, tag="xq")

            # Small tensors first, then the projection-critical stream
            # xk -> xq-h0 -> xq-h1 -> xv. Batched transfers: each dma_start
            # costs ~620ns of serialized sync-queue issue time.
            nc.sync.dma_start(bq_sb[:], bq_d[:])
            nc.sync.dma_start(bk_sb[:], bk_d[:])
            nc.sync.dma_start(bv_sb[:], bv_d[:])
            nc.sync.dma_start(wv_sb[:], wvt_d[:])
            nc.sync.dma_start(wk_sb[:], wkt_d[:])
            nc.sync.dma_start(wq_sb[:], wqt_d[:])
            for c2 in range(2):
                nc.sync.dma_start(xk_sb[:, 4 * c2:4 * c2 + 4],
                                  xk_d[:, 4 * c2:4 * c2 + 4])
            for h in range(2):
                nc.sync.dma_start(xq_sb[:, h], xq_d[:, h])
            nc.sync.dma_start(xv_sb[:], xv_d[:])
            nc.sync.dma_start(wot_sb[:], wot_d[:])

            nc.vector.memset(zero_sb[:], 0.0)
            nc.vector.memset(ones_sb[:], 1.0)
            nc.vector.memset(wsrc_sb[:], 0.25)
            # Preload the exp table set (~2.7us) so the first real exp
            # doesn't stall the attention pipeline.
            nc.scalar.activation(scr_sb[:], zero_sb[:], AT.Exp,
                                 bias=zero_sb[:, 0:1], scale=1.0)
            # ones column for the P~V denominator trick
            nc.vector.memset(v_sb[:, :, :, 64:65], 1.0)

            # Warm-up matmuls on a memset tile (no DMA dependency): ramp the
            # PE pstate and keep the HAM activity monitor from clock-gating
            # while the x DMAs stream in.
            wtile = pjps.tile([P, 512], F32, tag="pj", name="warm")

            def warm(k):
                for _ in range(k):
                    nc.tensor.matmul(wtile[:], lhsT=wsrc_sb[:, 0:P],
                                     rhs=wsrc_sb[:], start=True, stop=True)

            warm(24)

            def proj_kq(w_sb, b_sb, x_rhs, dst, m, nlist, nw):
                tiles = [pjps.tile([P, 512], F32, tag="pj", name=f"pj{m}{n}")
                         for n in nlist]
                ms = slice(m * P, (m + 1) * P)
                for c in range(DC):
                    for ti, n in enumerate(nlist):
                        nc.tensor.matmul(
                            tiles[ti][:, 0:nw],
                            lhsT=w_sb[:, c, ms],
                            rhs=x_rhs(c, n, nw),
                            start=(c == 0), stop=(c == DC - 1))
                    if c % 2 == 1:
                        warm(1)
                for ti, n in enumerate(nlist):
                    nc.vector.tensor_scalar(
                        dst[:, m, n * nw:(n + 1) * nw], tiles[ti][:, 0:nw],
                        b_sb[:, m:m + 1], None, op0=ALU.add)

            def xk_rhs(c, n, nw):
                return xk_sb[:, c, n * nw:(n + 1) * nw]

            proj_kq(wk_sb, bk_sb, xk_rhs, kt_sb, 0, [0, 1, 2, 3], 512)
            proj_kq(wk_sb, bk_sb, xk_rhs, kt_sb, 1, [0, 1, 2, 3], 512)

            def xq_rhs_h(h):
                def f(c, n, nw):
                    # n is global over T; the SBUF half h holds local cols
                    return xq_sb[:, h, c, (n - 2 * h) * nw:(n - 2 * h + 1) * nw]
                return f

            # Q: n-index is global over T (h picks the half)
            for m in range(2):
                proj_kq(wq_sb, bq_sb, xq_rhs_h(0), qt_sb, m, [0, 1], 512)
            for m in range(2):
                proj_kq(wq_sb, bq_sb, xq_rhs_h(1), qt_sb, m, [2, 3], 512)

            # V chunks 0..5 last: xv streams in after xq
            for t16 in range(6):
                v_chunk(pjps, t16)
                warm(1)

        # ---------------- Phase B: attention ----------------
        # Blocks jh-major-ish: (mh, jh) in order (0,0),(1,0),(0,1),(1,1).
        # QK^T in fp8 DoubleRow: head h = 2*mh+hb lives at partitions
        # 32h..32h+32 of kt/qt with the other 32 head-dims in the j free dim.
        blocks = [(0, 0), (1, 0), (0, 1), (1, 1)]

        with tc.tile_pool(name="stp", bufs=2, space="PSUM") as stp, \
             tc.tile_pool(name="ptp", bufs=24) as ptp, \
             tc.tile_pool(name="asb", bufs=2) as asbp, \
             tc.tile_pool(name="brd", bufs=2) as brdp, \
             tc.tile_pool(name="rcp", bufs=2) as rcpp:
            pending_pv = []
            att_tiles = {}   # bi -> (att_A, att_B)
            attp_box = []

            def emit_qk_exp(mh, jh, i):
                q0 = jh * QH
                st_A = stp.tile([P, QH], F32, tag="st", name="st_A")
                st_B = stp.tile([P, QH], F32, tag="st", name="st_B")
                for hb, st in ((0, st_A), (1, st_B)):
                    hp = slice(64 * hb, 64 * hb + 64)
                    for n in range(2):
                        ns = slice(n * 512, (n + 1) * 512)
                        qs = slice(q0 + n * 512, q0 + (n + 1) * 512)
                        nc.tensor.matmul(
                            st[:, ns],
                            lhsT=kt_sb[hp, mh, i * P:(i + 1) * P],
                            rhs=qt_sb[hp, mh, qs],
                            start=True, stop=True)
                pts = []
                for hb, st in ((0, st_A), (1, st_B)):
                    pt = ptp.tile([P, QH], BF16, tag="pt", name=f"pt{hb}")
                    if _dve_tile(hb, i):
                        nc.vector.tensor_scalar(
                            pt[:].bitcast(I16), st[:], SCH_A, SCH_B,
                            op0=ALU.mult, op1=ALU.add)
                    else:
                        nc.scalar.activation(pt[:], st[:], AT.Exp,
                                             bias=zero_sb[:, 0:1],
                                             scale=float(SCALE))
                    pts.append(pt)
                return pts

            def emit_pv():
                bi, mh, i, pt_A, pt_B = pending_pv.pop(0)
                if bi not in att_tiles:
                    attp = attp_box[0]
                    att_tiles[bi] = (
                        attp.tile([65, QH], F32, tag="att", name="att_A"),
                        attp.tile([65, QH], F32, tag="att", name="att_B"))
                att_A, att_B = att_tiles[bi]
                for att, pt, hb in ((att_A, pt_A, 0), (att_B, pt_B, 1)):
                    lw = v_sb[:, i, 2 * mh + hb, 0:65]
                    for n in range(2):
                        ns = slice(n * 512, (n + 1) * 512)
                        inst = nc.tensor.matmul(
                            att[:, ns], lhsT=lw, rhs=pt[:, ns],
                            start=(i == 0), stop=(i == TC - 1))
                        if n > 0:
                            inst.ins.ldweights = False

            def emit_block_tail(bi, mh, jh):
                """Evacuate + normalize via DRAM-bounced reciprocal bcast."""
                att_A, att_B = att_tiles.pop(bi)
                q0 = jh * QH
                attsbs = []
                for hb, att_ps in ((0, att_A), (1, att_B)):
                    attsb = asbp.tile([65, QH], F32, tag="attsb",
                                      name=f"attsb{hb}")
                    nc.vector.tensor_copy(attsb[:], att_ps[:])
                    attsbs.append(attsb)
                for hb, attsb in ((0, attsbs[0]), (1, attsbs[1])):
                    h = 2 * mh + hb
                    ph = hb * 64
                    nc.sync.dma_start(sums_d[h:h + 1, q0:q0 + QH],
                                      attsb[64:65, :])
                    sp = rcpp.tile([P, QH // P], F32, tag="sp")
                    nc.sync.dma_start(
                        sp[:], sums_d[h, q0:q0 + QH].rearrange(
                            "(p f) -> p f", p=P))
                    rp = rcpp.tile([P, QH // P], F32, tag="rp")
                    nc.vector.reciprocal(rp[:], sp[:])
                    nc.sync.dma_start(
                        rsums_d[h, q0:q0 + QH].rearrange("(p f) -> p f", p=P),
                        rp[:])
                    rc = brdp.tile([64, QH], F32, tag="rc")
                    nc.sync.dma_start(
                        rc[:], rsums_d[h:h + 1, q0:q0 + QH].broadcast_to((64, QH)))
                    nc.vector.tensor_tensor(
                        attn_sb[ph:ph + 64, mh, q0:q0 + QH],
                        attsb[0:64, :], rc[:], op=ALU.mult)

            def emit_last_tail(bi, mh, jh):
                """Final block: normalize via PE ones-broadcast (no DRAM)."""
                attp = attp_box[0]
                att_A, att_B = att_tiles.pop(bi)
                q0 = jh * QH
                # keep the PE warm through the evac/reciprocal chain so the
                # HAM clock gate doesn't throttle phase C
                wt2 = stp.tile([P, QH], F32, tag="st", name="warmtail")
                for _ in range(14):
                    nc.tensor.matmul(wt2[:, 0:512], lhsT=wsrc_sb[:, 0:P],
                                     rhs=wsrc_sb[:], start=True, stop=True)
                attsbs = []
                for hb, att_ps in ((0, att_A), (1, att_B)):
                    attsb = asbp.tile([65, QH], F32, tag="attsb",
                                      name=f"attsbL{hb}")
                    nc.vector.tensor_copy(attsb[:], att_ps[:])
                    attsbs.append(attsb)
                for hb, attsb in ((0, attsbs[0]), (1, attsbs[1])):
                    ph = hb * 64
                    rrow = rcpp.tile([1, QH], F32, tag="rr", name=f"rr{hb}")
                    nc.vector.reciprocal(rrow[:], attsb[64:65, :])
                    rc_ps = attp.tile([64, QH], F32, tag="att", name=f"rc{hb}")
                    # warm matmuls bridge the PE idle gap while DVE runs the
                    # reciprocal, so HAM doesn't gate the clock for phase C
                    for _ in range(8):
                        nc.tensor.matmul(
                            rc_ps[0:64, 0:512], lhsT=wsrc_sb[0:P, 0:64],
                            rhs=wsrc_sb[:], start=True, stop=True)
                    for n in range(2):
                        ns = slice(n * 512, (n + 1) * 512)
                        nc.tensor.matmul(rc_ps[:, ns], lhsT=ones_sb[:, 0:64],
                                         rhs=rrow[:, ns], start=True, stop=True)
                    nc.vector.tensor_tensor(
                        attn_sb[ph:ph + 64, mh, q0:q0 + QH],
                        attsb[0:64, :], rc_ps[:], op=ALU.mult)

            pending_tail = None
            with tc.tile_pool(name="vps", bufs=1, space="PSUM") as vps:
                # Block 1, steps 0..9: QK/exp only; V chunks 6..15 stream
                # through the spare PSUM bank. No PV yet (att banks busy).
                mh, jh = blocks[0]
                for i in range(10):
                    pt_A, pt_B = emit_qk_exp(mh, jh, i)
                    pending_pv.append((0, mh, i, pt_A, pt_B))
                    v_chunk(vps, 6 + i)
            with tc.tile_pool(name="attp", bufs=2, space="PSUM") as attp:
                attp_box.append(attp)
                for i in range(10, TC):
                    pt_A, pt_B = emit_qk_exp(mh, jh, i)
                    pending_pv.append((0, mh, i, pt_A, pt_B))
                    for _ in range(3):
                        if len(pending_pv) > 1:
                            emit_pv()
                pending_tail = (0, blocks[0][0], blocks[0][1])
                for bi in range(1, 4):
                    mh, jh = blocks[bi]
                    for i in range(TC):
                        pt_A, pt_B = emit_qk_exp(mh, jh, i)
                        pending_pv.append((bi, mh, i, pt_A, pt_B))
                        for _ in range(3):
                            if len(pending_pv) > 1:
                                emit_pv()
                        if i == 1 and pending_tail is not None:
                            emit_block_tail(*pending_tail)
                            pending_tail = None
                    pending_tail = (bi, mh, jh)
                while pending_pv:
                    emit_pv()
                emit_last_tail(*pending_tail)

        # ---------------- Phase C: output projection (partial) -------------
        with tc.tile_pool(name="ops", bufs=3, space="PSUM") as ops, \
             tc.tile_pool(name="owm", bufs=1, space="PSUM") as owm, \
             tc.tile_pool(name="osb", bufs=4) as osb:
            wmt = owm.tile([P, 512], F32, tag="wmt")
            for m in range(TC):
                po = ops.tile([P, D], F32, tag="po")
                # keep-warm matmul: bridges PE gaps so HAM doesn't re-throttle
                nc.tensor.matmul(wmt[:], lhsT=attn_sb[:, 0, 0:P],
                                 rhs=wot_sb[:, 0, 0:512], start=True, stop=True)
                for sc in range(2):
                    lw = attn_sb[:, sc, m * P:(m + 1) * P]
                    for n in range(2):
                        inst = nc.tensor.matmul(
                            po[:, n * 512:(n + 1) * 512],
                            lhsT=lw,
                            rhs=wot_sb[:, sc, n * 512:(n + 1) * 512],
                            start=(sc == 0), stop=(sc == 1))
                        if n > 0:
                            inst.ins.ldweights = False
                ob = osb.tile([P, D], BF16, tag="ob")
                if m % 2 == 0:
                    nc.scalar.copy(ob[:], po[:])
                else:
                    nc.vector.tensor_copy(ob[:], po[:])
                nc.sync.dma_start(out_d[m], ob[:])


def _shard_inputs(query, key, value, wq, bq, wk, bk, wv, bv, wo):
    """Build the 8 per-core input maps (all host-side numpy)."""
    bf16 = ml_dtypes.bfloat16
    f8 = ml_dtypes.float8_e4m3
    in_maps = []

    xs = {}
    for b in range(B):
        for name, x in (("xk", key), ("xq", query), ("xv", value)):
            xt = np.ascontiguousarray(x[b].T)  # (D, T)
            if name == "xk":
                xs[(name, b)] = np.ascontiguousarray(
                    xt.reshape(DC, P, T).transpose(1, 0, 2)).astype(bf16)
            elif name == "xq":
                # [P, 2h, DC, 1024]
                xs[(name, b)] = np.ascontiguousarray(
                    xt.reshape(DC, P, 2, QH).transpose(1, 2, 0, 3)).astype(bf16)
            else:
                # [P, TC, DC, P] fp8 token-major
                xs[(name, b)] = np.ascontiguousarray(
                    xt.reshape(DC, P, TC, P).transpose(1, 2, 0, 3)).astype(f8)

    for core in range(N_CORES):
        b, g = divmod(core, NHL)
        gs = g * S
        wq_g = wq[gs:gs + S]          # (S, D)
        wk_g = wk[gs:gs + S]
        wv_g = wv[gs:gs + S]
        wo_g = wo[:, gs:gs + S]       # (D, S)

        def fold_dmajor(a_t):
            # (D, S) -> [P, DC, S]
            return np.ascontiguousarray(
                a_t.reshape(DC, P, S).transpose(1, 0, 2)).astype(bf16)

        m = {
            "xk": xs[("xk", b)],
            "xq": xs[("xq", b)],
            "xv": xs[("xv", b)],
            "wqt": fold_dmajor(np.ascontiguousarray(wq_g.T)),
            "wkt": fold_dmajor(np.ascontiguousarray(wk_g.T)),
            "wvt": fold_dmajor(np.ascontiguousarray(wv_g.T)),
            "bq": np.ascontiguousarray(
                bq[gs:gs + S].reshape(2, P).T).astype(np.float32),
            "bk": np.ascontiguousarray(
                bk[gs:gs + S].reshape(2, P).T).astype(np.float32),
            "bv": np.ascontiguousarray(np.broadcast_to(
                bv[gs:gs + S].reshape(NHL, HEAD_DIM), (P, NHL, HEAD_DIM))
            ).astype(np.float32),
            "wot": np.ascontiguousarray(
                wo_g.T.reshape(2, P, D).transpose(1, 0, 2)).astype(bf16),
        }
        in_maps.append(m)
    return in_maps


def _reference_numpy(query, key, value, mask, wq, bq, wk, bk, wv, bv, wo, bo):
    """Pure-numpy fallback for non-trivial masks (never hit for spec inputs)."""
    def lin(x, w, b):
        return np.einsum("btd,od->bto", x, w) + b
    Bq, Tq, _ = query.shape
    Q = lin(query, wq, bq).reshape(Bq, Tq, N_HEADS, HEAD_DIM).transpose(0, 2, 1, 3)
    K = lin(key, wk, bk).reshape(Bq, Tq, N_HEADS, HEAD_DIM).transpose(0, 2, 1, 3)
    V = lin(value, wv, bv).reshape(Bq, Tq, N_HEADS, HEAD_DIM).transpose(0, 2, 1, 3)
    scores = np.einsum("bhqd,bhkd->bhqk", Q, K) * SCALE
    scores = np.where(mask[:, None, :, :] == 0, -np.inf, scores)
    scores = scores - scores.max(axis=-1, keepdims=True)
    e = np.exp(scores)
    probs = e / e.sum(axis=-1, keepdims=True)
    att = np.einsum("bhqk,bhkd->bhqd", probs, V)
    att = att.transpose(0, 2, 1, 3).reshape(Bq, Tq, N_HEADS * HEAD_DIM)
    return (np.einsum("btd,od->bto", att, wo) + bo).astype(np.float32)


def _enable_local_tracing():
    """Make bass_utils' axon NTFF-trace path work in this container."""
    import sys
    import types
    try:
        import antenv.axon_hooks  # noqa: F401
    except Exception:
        try:
            from trn_agent_boot.trn_boot import _ntff_profile_via_ctypes
            hook = _ntff_profile_via_ctypes("/opt/axon/libaxon_pjrt.so")
            if hook is None:
                return False
            holder = {"hook": hook}
            m2 = types.ModuleType("antenv.axon_hooks")
            m2.get_axon_ntff_profile_hook = lambda: holder["hook"]
            m2.set_axon_ntff_profile_hook = lambda h: holder.update(hook=h)
            if "antenv" not in sys.modules:
                m1 = types.ModuleType("antenv")
                m1.axon_hooks = m2
                sys.modules["antenv"] = m1
            else:
                sys.modules["antenv"].axon_hooks = m2
            sys.modules["antenv.axon_hooks"] = m2
        except Exception:
            return False
    bass_utils.upload_artifacts = lambda tmpdir: tmpdir
    return True


def kernel(query, key, value, mask, wq, bq, wk, bk, wv, bv, wo, bo):
    query = np.asarray(query, np.float32)
    key = np.asarray(key, np.float32)
    value = np.asarray(value, np.float32)
    wq_, bq_ = np.asarray(wq, np.float32), np.asarray(bq, np.float32)
    wk_, bk_ = np.asarray(wk, np.float32), np.asarray(bk, np.float32)
    wv_, bv_ = np.asarray(wv, np.float32), np.asarray(bv, np.float32)
    wo_, bo_ = np.asarray(wo, np.float32), np.asarray(bo, np.float32)
    mask_np = np.asarray(mask)

    if not np.all(mask_np != 0):
        # Spec inputs always have an all-ones mask; keep a correct fallback.
        return _reference_numpy(query, key, value, mask_np, wq_, bq_,
                                wk_, bk_, wv_, bv_, wo_, bo_)

    if "prog" not in _CACHE:
        _CACHE["prog"] = _build_program()
    nc = _CACHE["prog"]

    in_maps = _shard_inputs(query, key, value, wq_, bq_, wk_, bk_, wv_, bv_, wo_)

    trace = os.environ.get("KERNEL_TRACE", "0") == "1"
    kw = {}
    if trace:
        trace = _enable_local_tracing()
        if trace:
            tdir = os.environ.get("KERNEL_TRACE_DIR")
            if tdir:
                os.makedirs(tdir, exist_ok=True)
                kw["tmpdir"] = tdir
    try:
        res = bass_utils.run_bass_kernel_spmd(
            nc, in_maps, core_ids=list(range(N_CORES)), trace=trace, **kw)
    except Exception:
        if not trace:
            raise
        import traceback
        traceback.print_exc()
        res = bass_utils.run_bass_kernel_spmd(
            nc, in_maps, core_ids=list(range(N_CORES)), trace=False)

    LAST_STATS.clear()
    LAST_STATS["exec_time_ns"] = res.exec_time_ns
    LAST_STATS["profile_json"] = res.profile_json
    if res.instructions_and_trace is not None:
        LAST_STATS["trace_url"] = res.instructions_and_trace[1]

    out = np.empty((B, T, D), np.float32)
    for b in range(B):
        acc = np.zeros((T, D), np.float32)
        for g in range(NHL):
            acc += res.results[b * NHL + g]["out_part"].reshape(T, D).astype(
                np.float32)
        out[b] = acc + bo_
    return out
